# revision 11
# baseline (speedup 1.0000x reference)
"""Multi-head self-attention (diag-zero mask) TRN2 kernel, 8-core head-parallel.

Sharding: 16 heads / 8 cores = 2 heads per core; every core sees the full
sequence (both batches), computes Q/K/V projections for its 2 heads,
attention, and its partial out-projection (Wo rows for its head block).
Host sums the 8 partial outputs (the out_proj all-reduce) and adds biases.

Math notes:
  - 1/sqrt(Dh) folded into Wq/bq on host.
  - scores are computed transposed (keys on partitions, queries on free dim)
    so exp() needs no on-chip reduction; the softmax denominator Z comes for
    free from an appended ones-column on V in the A@V matmul.
  - diag-zero mask: multiply the score diagonal block by (1-eye) before exp
    (masked score 0 -> exp(0) = 1, matching the reference softmax).
  - bv and bo contributions are rank-1/constant terms folded in on host:
    out += bv @ Wo.T + bo.

Schedule (v2): the ACT engine (exp) is the bottleneck at ~122us/core, so all
PE-only work is spliced into the attention stream's PE slack:
  proj(b0) -> attn(b0) [proj(b1) spliced per qt] ->
  attn(b1) [outproj(b0) + outproj(b1,qt-1) spliced per qt] -> outproj tail.
Out-projection partials are emitted in bf16 (summed on host in f64).
"""

from contextlib import ExitStack

import numpy as np
import ml_dtypes

import concourse.bass as bass
import concourse.tile as tile
from concourse import bacc, mybir
from concourse.bass_utils import run_bass_kernel_spmd

BF16 = mybir.dt.bfloat16
F32 = mybir.dt.float32

B = 2
D = 1024
H = 16
DH = 64
NCORES = 8
HLOC = H // NCORES          # 2 heads per core
DLOC = HLOC * DH            # 128 head-dims per core
KC = D // 128               # 8 contraction chunks for projections
MMW = 512                   # matmul moving width (one PSUM bank of f32)


def emit_kernel(tc, M, xT, wqT, wkT, wvT, woT, bq, bk, mask, out, dbg=None):
    """Emit the per-core program. M = per-batch sequence length."""
    nc = tc.nc
    S = B * M               # flattened sequence rows
    NKT = M // 128          # key tiles per batch
    NQT = M // MMW          # 512-wide q tiles per batch

    with ExitStack() as ctx:
        consts = ctx.enter_context(tc.tile_pool(name="consts", bufs=1))
        QT = consts.tile([128, S], BF16)    # [2 heads x 64 dims, S]
        KT = consts.tile([128, S], BF16)
        V1 = consts.tile([128, B, NKT, HLOC, 65], BF16)  # V natural + ones col
        Wo_sb = consts.tile([128, D], BF16)
        bq_sb = consts.tile([128, 1], F32)
        bk_sb = consts.tile([128, 1], F32)
        mask_sb = consts.tile([128, 128], F32)

        nc.sync.dma_start(Wo_sb, woT.ap())
        nc.sync.dma_start(bq_sb, bq.ap())
        nc.sync.dma_start(bk_sb, bk.ap())
        nc.sync.dma_start(mask_sb, mask.ap())
        nc.vector.memset(V1[:, :, :, :, 64:65], 1.0)

        # attention pools (live for the whole kernel)
        stp = ctx.enter_context(tc.tile_pool(name="st_psum", bufs=5, space="PSUM"))
        atp = ctx.enter_context(tc.tile_pool(name="at_pool", bufs=2 * NKT + 4))
        ctp = ctx.enter_context(tc.tile_pool(name="ct_psum", bufs=2, space="PSUM"))
        rzp = ctx.enter_context(tc.tile_pool(name="rz_pool", bufs=4))
        rzbp = ctx.enter_context(tc.tile_pool(name="rzb_pool", bufs=4))
        cp = ctx.enter_context(tc.tile_pool(name="c_pool", bufs=B))
        C_tiles = {}

        # projection pools (live until proj(b1) done, i.e. through attn(b0))
        proj_sbuf = ctx.enter_context(tc.tile_pool(name="proj_sbuf", bufs=1))
        vstg_pool = ctx.enter_context(tc.tile_pool(name="vstg_pool", bufs=4))

        Wq_sb = proj_sbuf.tile([128, KC, DLOC], BF16)
        Wk_sb = proj_sbuf.tile([128, KC, DLOC], BF16)
        Wv_sb = proj_sbuf.tile([128, KC, DLOC], BF16)
        nc.sync.dma_start(Wk_sb, wkT.ap().rearrange("(c p) d -> p c d", p=128))
        nc.sync.dma_start(Wv_sb, wvT.ap().rearrange("(c p) d -> p c d", p=128))
        nc.sync.dma_start(Wq_sb, wqT.ap().rearrange("(c p) d -> p c d", p=128))
        X = proj_sbuf.tile([128, KC, S], BF16)
        xT_r = xT.ap().rearrange("(c p) m -> p c m", p=128)
        for mt in range(S // MMW):
            nc.sync.dma_start(
                X[:, :, mt * MMW:(mt + 1) * MMW],
                xT_r[:, :, mt * MMW:(mt + 1) * MMW],
            )
        VT = proj_sbuf.tile([128, S], BF16)

        def proj_tile(b, mt_loc, which, pp):
            """One projection (K, V or Q) for one 512-wide m-tile of batch b.
            For V, also the transpose staging for that tile's 4 key blocks."""
            mt = b * NQT + mt_loc
            W_sb, dst, bias_sb = {
                "k": (Wk_sb, KT, bk_sb),
                "v": (Wv_sb, VT, None),
                "q": (Wq_sb, QT, bq_sb),
            }[which]
            ps = pp.tile([128, MMW], F32, name="proj_ps")
            for kc in range(KC):
                nc.tensor.matmul(
                    ps,
                    lhsT=W_sb[:, kc, :],
                    rhs=X[:, kc, mt * MMW:(mt + 1) * MMW],
                    start=(kc == 0),
                    stop=(kc == KC - 1),
                )
            dslice = dst[:, mt * MMW:(mt + 1) * MMW]
            if bias_sb is not None:
                nc.vector.tensor_scalar_add(dslice, ps, bias_sb)
            else:
                nc.vector.tensor_copy(dslice, ps)
            if dst is VT:
                # V natural layout: DMA transpose into aligned staging
                # tiles (xbar transpose corrupts data when the dst is
                # an unaligned slice of a larger tile), then DVE copy.
                for tl in range(MMW // 128):
                    t = mt_loc * (MMW // 128) + tl
                    vs = vstg_pool.tile([128, 128], BF16, name="vstg")
                    nc.sync.dma_start(
                        vs,
                        VT[:, b * M + t * 128: b * M + (t + 1) * 128],
                        transpose=True,
                    )
                    for h in range(HLOC):
                        nc.vector.tensor_copy(
                            V1[:, b, t, h, 0:64], vs[:, h * 64:(h + 1) * 64]
                        )

        def proj_chunk(b, mt_loc, pp):
            for which in ("k", "v", "q"):
                proj_tile(b, mt_loc, which, pp)

        def attn_qt(b, qt, filler=None):
            """One 512-query tile of attention for batch b. `filler` is a
            deque of small thunks emitting PE work to fill the exp-lag gap;
            they are spread evenly across the score/AV stream."""
            C = C_tiles[b]
            q0 = b * M + qt * MMW
            at_tiles = {}
            cts = [ctp.tile([65, MMW], F32, name="ct") for _ in range(HLOC)]

            def av(kt):
                for h in range(HLOC):
                    nc.tensor.matmul(
                        cts[h],
                        lhsT=V1[:, b, kt, h, :],
                        rhs=at_tiles[kt, h],
                        start=(kt == 0),
                        stop=(kt == NKT - 1),
                    )

            nfil = len(filler) if filler else 0
            shift = min(3, NKT - 1)
            for kt in range(NKT):
                # paired-head score tiles: one PSUM bank per head, emitted
                # back-to-back so the PE runs the K=64 pair concurrently
                sts = []
                for h in range(HLOC):
                    hs = slice(h * 64, (h + 1) * 64)
                    st = stp.tile([128, MMW], F32, name="st")
                    sts.append(st)
                    nc.tensor.matmul(
                        st,
                        lhsT=KT[hs, b * M + kt * 128: b * M + (kt + 1) * 128],
                        rhs=QT[hs, q0: q0 + MMW],
                        start=True,
                        stop=True,
                    )
                c0 = kt * 128 - qt * MMW
                if 0 <= c0 < MMW:
                    for h in range(HLOC):
                        nc.vector.tensor_mul(
                            sts[h][:, c0:c0 + 128], sts[h][:, c0:c0 + 128],
                            mask_sb,
                        )
                for h in range(HLOC):
                    at = atp.tile([128, MMW], BF16, name="at")
                    nc.scalar.activation(at, sts[h],
                                         mybir.ActivationFunctionType.Exp)
                    at_tiles[kt, h] = at
                # interleave A@V of an already-exp'd tile between score
                # pairs: keeps the PE busy without pausing the exp stream
                if kt >= shift:
                    av(kt - shift)
                # spread filler PE work evenly across the qt; the exp stream
                # (ACT) is the bottleneck, this rides in PE slack
                if filler and (kt + 1) * nfil // NKT > kt * nfil // NKT:
                    filler.popleft()()
            for kt in range(NKT - shift, NKT):
                av(kt)
            while filler:
                filler.popleft()()
            for h, ct in enumerate(cts):
                rz = rzp.tile([1, MMW], F32, name="rz")
                nc.vector.reciprocal(rz, ct[64:65, :])
                rzb = rzbp.tile([64, MMW], F32, name="rzb")
                nc.gpsimd.partition_broadcast(rzb, rz)
                nc.vector.tensor_mul(
                    C[h * 64:(h + 1) * 64, qt * MMW:(qt + 1) * MMW],
                    ct[0:64, :], rzb,
                )

        def outproj_unit(b, mt, j, opp, osp):
            """One 128-row x 512-col out-projection tile of batch b."""
            C = C_tiles[b]
            op = opp.tile([128, MMW], F32, name="op")
            nc.tensor.matmul(
                op,
                lhsT=C[:, mt * 128:(mt + 1) * 128],
                rhs=Wo_sb[:, j * MMW:(j + 1) * MMW],
                start=True,
                stop=True,
            )
            osb = osp.tile([128, MMW], BF16, name="osb")
            nc.vector.tensor_copy(osb, op)
            nc.sync.dma_start(
                out.ap()[b * M + mt * 128: b * M + (mt + 1) * 128,
                         j * MMW:(j + 1) * MMW],
                osb,
            )

        def outproj_chunk(b, mts, opp, osp):
            for mt in mts:
                for j in range(D // MMW):
                    outproj_unit(b, mt, j, opp, osp)

        # ---------------- the pipelined schedule ----------------
        from collections import deque
        import kernel as _self
        skip_attn = getattr(_self, "ABL_skip_attn", False)
        skip_outproj = getattr(_self, "ABL_skip_outproj", False)
        no_splice = getattr(_self, "ABL_no_splice", False)

        for b in range(B):
            C_tiles[b] = cp.tile([128, M], BF16, name="C")

        with tc.tile_pool(name="proj_psum", bufs=1, space="PSUM") as pp:
            # serial head: projections for batch 0
            for mt_loc in range(NQT):
                proj_chunk(0, mt_loc, pp)
            if skip_attn:
                for mt_loc in range(NQT):
                    proj_chunk(1, mt_loc, pp)
                nc.sync.dma_start(out.ap()[0:128, 0:D], QT[:, 0:D])
                return
            # attention(b0) with proj(b1) spliced into the per-qt PE slack
            for qt in range(NQT):
                if no_splice:
                    attn_qt(0, qt)
                else:
                    attn_qt(0, qt, filler=deque(
                        (lambda ml=qt, w=w: proj_tile(1, ml, w, pp))
                        for w in ("k", "v", "q")))
            if no_splice:
                for mt_loc in range(NQT):
                    proj_chunk(1, mt_loc, pp)

        if skip_outproj:
            for qt in range(NQT):
                attn_qt(1, qt)
            nc.sync.dma_start(out.ap()[0:128, 0:D], C_tiles[0][:, 0:D])
            nc.sync.dma_start(out.ap()[128:256, 0:D], C_tiles[1][:, 0:D])
            return

        with tc.tile_pool(name="op_psum", bufs=1, space="PSUM") as opp, \
             tc.tile_pool(name="out_sbuf", bufs=4) as osp:
            NMT = M // 128
            MPQ = NMT // NQT        # 128-row out-proj tiles per qt slot
            NJ = D // MMW
            # attention(b1) with outproj(b0) + outproj(b1, qt-1) spliced
            for qt in range(NQT):
                fillers = deque(
                    (lambda mt=mt, j=j: outproj_unit(0, mt, j, opp, osp))
                    for mt in range(qt * MPQ, (qt + 1) * MPQ)
                    for j in range(NJ))
                if qt > 0:
                    fillers.extend(
                        (lambda mt=mt, j=j: outproj_unit(1, mt, j, opp, osp))
                        for mt in range((qt - 1) * MPQ, qt * MPQ)
                        for j in range(NJ))
                attn_qt(1, qt, filler=None if no_splice else fillers)
            if no_splice:
                for mt in range(NMT):
                    outproj_chunk(0, (mt,), opp, osp)
                    outproj_chunk(1, (mt,), opp, osp)
            else:
                # tail: final out-proj chunk of batch 1
                outproj_chunk(1, tuple(range((NQT - 1) * MPQ, NMT)), opp, osp)

        if dbg is not None:
            nc.sync.dma_start(dbg["qt"].ap(), QT)
            nc.sync.dma_start(dbg["kt"].ap(), KT)
            nc.sync.dma_start(dbg["v1"].ap(), V1.rearrange("p a b c d -> p (a b c d)"))
            nc.sync.dma_start(dbg["c"].ap()[:, 0:M], C_tiles[0])
            nc.sync.dma_start(dbg["c"].ap()[:, M:S], C_tiles[1])


def build_bass(M, debug=False, reps=1):
    """Build + compile the per-core Bass program (same program on all cores).

    reps > 1 wraps the whole body in an on-device loop — used only for
    timing (amortizes host dispatch overhead over many executions).
    """
    S = B * M
    nc = bacc.Bacc("TRN2", target_bir_lowering=False, debug=False)
    xT = nc.dram_tensor("xT", [D, S], BF16, kind="ExternalInput")
    wqT = nc.dram_tensor("wqT", [D, DLOC], BF16, kind="ExternalInput")
    wkT = nc.dram_tensor("wkT", [D, DLOC], BF16, kind="ExternalInput")
    wvT = nc.dram_tensor("wvT", [D, DLOC], BF16, kind="ExternalInput")
    woT = nc.dram_tensor("woT", [DLOC, D], BF16, kind="ExternalInput")
    bq = nc.dram_tensor("bq", [DLOC, 1], F32, kind="ExternalInput")
    bk = nc.dram_tensor("bk", [DLOC, 1], F32, kind="ExternalInput")
    mask = nc.dram_tensor("mask", [128, 128], F32, kind="ExternalInput")
    out = nc.dram_tensor("out", [S, D], BF16, kind="ExternalOutput")

    dbg = None
    if debug:
        NKT = M // 128
        dbg = {
            "qt": nc.dram_tensor("dbg_qt", [128, S], BF16, kind="ExternalOutput"),
            "kt": nc.dram_tensor("dbg_kt", [128, S], BF16, kind="ExternalOutput"),
            "v1": nc.dram_tensor("dbg_v1", [128, B * NKT * HLOC * 65], BF16,
                                 kind="ExternalOutput"),
            "c": nc.dram_tensor("dbg_c", [128, S], BF16, kind="ExternalOutput"),
        }

    with tile.TileContext(nc) as tc:
        if reps > 1:
            with tc.For_i(0, reps, 1):
                emit_kernel(tc, M, xT, wqT, wkT, wvT, woT, bq, bk, mask, out,
                            dbg=dbg)
        else:
            emit_kernel(tc, M, xT, wqT, wkT, wvT, woT, bq, bk, mask, out,
                        dbg=dbg)
    nc.compile()
    return nc


def make_in_maps(M, x, Wq, bq, Wk, bk, Wv, Wo):
    """Host-side sharding: per-core input dicts."""
    S = B * M
    bf = ml_dtypes.bfloat16
    scale = 1.0 / np.sqrt(DH)
    xT = np.ascontiguousarray(x.reshape(S, D).T).astype(bf)
    mask = (1.0 - np.eye(128, dtype=np.float32))
    in_maps = []
    for c in range(NCORES):
        sl = slice(c * DLOC, (c + 1) * DLOC)
        in_maps.append({
            "xT": xT,
            "wqT": np.ascontiguousarray((Wq[sl] * scale).T).astype(bf),
            "wkT": np.ascontiguousarray(Wk[sl].T).astype(bf),
            "wvT": np.ascontiguousarray(Wv[sl].T).astype(bf),
            "woT": np.ascontiguousarray(Wo[:, sl].T).astype(bf),
            "bq": (bq[sl] * scale).reshape(DLOC, 1).astype(np.float32),
            "bk": bk[sl].reshape(DLOC, 1).astype(np.float32),
            "mask": mask,
        })
    return in_maps


_NC_CACHE = {}


def kernel(x, Wq, bq, Wk, bk, Wv, bv, Wo, bo):
    x = np.asarray(x, dtype=np.float32)
    Wq = np.asarray(Wq, dtype=np.float32)
    bq = np.asarray(bq, dtype=np.float32)
    Wk = np.asarray(Wk, dtype=np.float32)
    bk = np.asarray(bk, dtype=np.float32)
    Wv = np.asarray(Wv, dtype=np.float32)
    bv = np.asarray(bv, dtype=np.float32)
    Wo = np.asarray(Wo, dtype=np.float32)
    bo = np.asarray(bo, dtype=np.float32)

    M = x.shape[1]
    if M not in _NC_CACHE:
        _NC_CACHE[M] = build_bass(M)
    nc = _NC_CACHE[M]

    in_maps = make_in_maps(M, x, Wq, bq, Wk, bk, Wv, Wo)
    res = run_bass_kernel_spmd(nc, in_maps, core_ids=list(range(NCORES)))

    out = np.zeros((B * M, D), np.float64)
    for c in range(NCORES):
        out += res.results[c]["out"].astype(np.float64)
    out = out.astype(np.float32)
    out += bv @ Wo.T + bo          # folded bv/bo contribution
    return out.reshape(B, M, D)


# revision 13
# speedup vs baseline: 1.4779x; 1.4779x over previous
"""Multi-head self-attention (diag-zero mask) TRN2 kernel, 8-core head-parallel.

Sharding: 16 heads / 8 cores = 2 heads per core; every core sees the full
sequence (both batches), computes Q/K/V projections for its 2 heads,
attention, and its partial out-projection (Wo rows for its head block).
Host sums the 8 partial outputs (the out_proj all-reduce) and adds biases.

Math notes:
  - 1/sqrt(Dh) folded into Wq/bq on host.
  - scores are computed transposed (keys on partitions, queries on free dim)
    so exp() needs no on-chip reduction; the softmax denominator Z comes for
    free from an appended ones-column on V in the A@V matmul.
  - diag-zero mask: multiply the score diagonal block by (1-eye) before exp
    (masked score 0 -> exp(0) = 1, matching the reference softmax).
  - bv and bo contributions are rank-1/constant terms folded in on host:
    out += bv @ Wo.T + bo.

Schedule (v2): the ACT engine (exp) is the bottleneck at ~122us/core, so all
PE-only work is spliced into the attention stream's PE slack:
  proj(b0) -> attn(b0) [proj(b1) spliced per qt] ->
  attn(b1) [outproj(b0) + outproj(b1,qt-1) spliced per qt] -> outproj tail.
Out-projection partials are emitted in bf16 (summed on host in f64).
"""

from contextlib import ExitStack

import numpy as np
import ml_dtypes

import concourse.bass as bass
import concourse.tile as tile
from concourse import bacc, mybir
from concourse.bass_utils import run_bass_kernel_spmd

BF16 = mybir.dt.bfloat16
F32 = mybir.dt.float32

B = 2
D = 1024
H = 16
DH = 64
NCORES = 8
HLOC = H // NCORES          # 2 heads per core
DLOC = HLOC * DH            # 128 head-dims per core
KC = D // 128               # 8 contraction chunks for projections
MMW = 512                   # matmul moving width (one PSUM bank of f32)


def emit_kernel(tc, M, xT, wqT, wkT, wvT, woT, bq, bk, mask, out, dbg=None):
    """Emit the per-core program. M = per-batch sequence length."""
    nc = tc.nc
    S = B * M               # flattened sequence rows
    NKT = M // 128          # key tiles per batch
    NQT = M // MMW          # 512-wide q tiles per batch

    with ExitStack() as ctx:
        consts = ctx.enter_context(tc.tile_pool(name="consts", bufs=1))
        QT = consts.tile([128, S], BF16)    # [2 heads x 64 dims, S]
        KT = consts.tile([128, S], BF16)
        V1 = consts.tile([128, B, NKT, HLOC, 65], BF16)  # V natural + ones col
        Wo_sb = consts.tile([128, D], BF16)
        bq_sb = consts.tile([128, 1], F32)
        bk_sb = consts.tile([128, 1], F32)
        mask_sb = consts.tile([128, 128], F32)

        nc.sync.dma_start(Wo_sb, woT.ap())
        nc.sync.dma_start(bq_sb, bq.ap())
        nc.sync.dma_start(bk_sb, bk.ap())
        nc.sync.dma_start(mask_sb, mask.ap())
        nc.vector.memset(V1[:, :, :, :, 64:65], 1.0)

        # attention pools (live for the whole kernel)
        stp = ctx.enter_context(tc.tile_pool(name="st_psum", bufs=2, space="PSUM"))
        atp = ctx.enter_context(tc.tile_pool(name="at_pool", bufs=2 * NKT + 4))
        ctp = ctx.enter_context(tc.tile_pool(name="ct_psum", bufs=2, space="PSUM"))
        rzp = ctx.enter_context(tc.tile_pool(name="rz_pool", bufs=4))
        rzbp = ctx.enter_context(tc.tile_pool(name="rzb_pool", bufs=4))
        cp = ctx.enter_context(tc.tile_pool(name="c_pool", bufs=B))
        C_tiles = {}

        # projection pools (live until proj(b1) done, i.e. through attn(b0))
        proj_sbuf = ctx.enter_context(tc.tile_pool(name="proj_sbuf", bufs=1))
        vstg_pool = ctx.enter_context(tc.tile_pool(name="vstg_pool", bufs=4))

        Wq_sb = proj_sbuf.tile([128, KC, DLOC], BF16)
        Wk_sb = proj_sbuf.tile([128, KC, DLOC], BF16)
        Wv_sb = proj_sbuf.tile([128, KC, DLOC], BF16)
        nc.sync.dma_start(Wk_sb, wkT.ap().rearrange("(c p) d -> p c d", p=128))
        nc.sync.dma_start(Wv_sb, wvT.ap().rearrange("(c p) d -> p c d", p=128))
        nc.sync.dma_start(Wq_sb, wqT.ap().rearrange("(c p) d -> p c d", p=128))
        X = proj_sbuf.tile([128, KC, S], BF16)
        xT_r = xT.ap().rearrange("(c p) m -> p c m", p=128)
        for mt in range(S // MMW):
            nc.sync.dma_start(
                X[:, :, mt * MMW:(mt + 1) * MMW],
                xT_r[:, :, mt * MMW:(mt + 1) * MMW],
            )
        VT = proj_sbuf.tile([128, S], BF16)

        def proj_tile(b, mt_loc, which, pp):
            """One projection (K, V or Q) for one 512-wide m-tile of batch b.
            For V, also the transpose staging for that tile's 4 key blocks."""
            mt = b * NQT + mt_loc
            W_sb, dst, bias_sb = {
                "k": (Wk_sb, KT, bk_sb),
                "v": (Wv_sb, VT, None),
                "q": (Wq_sb, QT, bq_sb),
            }[which]
            ps = pp.tile([128, MMW], F32, name="proj_ps")
            for kc in range(KC):
                nc.tensor.matmul(
                    ps,
                    lhsT=W_sb[:, kc, :],
                    rhs=X[:, kc, mt * MMW:(mt + 1) * MMW],
                    start=(kc == 0),
                    stop=(kc == KC - 1),
                )
            dslice = dst[:, mt * MMW:(mt + 1) * MMW]
            if bias_sb is not None:
                nc.vector.tensor_scalar_add(dslice, ps, bias_sb)
            else:
                nc.vector.tensor_copy(dslice, ps)
            if dst is VT:
                # V natural layout: DMA transpose into aligned staging
                # tiles (xbar transpose corrupts data when the dst is
                # an unaligned slice of a larger tile), then DVE copy.
                for tl in range(MMW // 128):
                    t = mt_loc * (MMW // 128) + tl
                    vs = vstg_pool.tile([128, 128], BF16, name="vstg")
                    nc.sync.dma_start(
                        vs,
                        VT[:, b * M + t * 128: b * M + (t + 1) * 128],
                        transpose=True,
                    )
                    for h in range(HLOC):
                        nc.vector.tensor_copy(
                            V1[:, b, t, h, 0:64], vs[:, h * 64:(h + 1) * 64]
                        )

        def proj_chunk(b, mt_loc, pp):
            for which in ("k", "v", "q"):
                proj_tile(b, mt_loc, which, pp)

        def attn_qt(b, qt, filler=None):
            """One 512-query tile of attention for batch b. `filler` is a
            deque of small thunks emitting PE work to fill the exp-lag gap;
            they are spread evenly across the score/AV stream."""
            C = C_tiles[b]
            q0 = b * M + qt * MMW
            at_tiles = {}
            cts = [ctp.tile([65, MMW], F32, name="ct") for _ in range(HLOC)]

            def av(kt):
                for h in range(HLOC):
                    nc.tensor.matmul(
                        cts[h],
                        lhsT=V1[:, b, kt, h, :],
                        rhs=at_tiles[kt, h],
                        start=(kt == 0),
                        stop=(kt == NKT - 1),
                    )

            nfil = len(filler) if filler else 0
            shift = min(2, NKT - 1)
            for kt in range(NKT):
                # paired-head score tile: one [128,1024] PSUM tile per kt,
                # halves written per head (K=64 pair runs concurrently on the
                # PE); exp per half so subtile WAR tracking releases each
                # half early — doubles the effective st pipeline depth.
                st = stp.tile([128, HLOC * MMW], F32, name="st")
                for h in range(HLOC):
                    hs = slice(h * 64, (h + 1) * 64)
                    nc.tensor.matmul(
                        st[:, h * MMW:(h + 1) * MMW],
                        lhsT=KT[hs, b * M + kt * 128: b * M + (kt + 1) * 128],
                        rhs=QT[hs, q0: q0 + MMW],
                        start=True,
                        stop=True,
                    )
                c0 = kt * 128 - qt * MMW
                if 0 <= c0 < MMW:
                    for h in range(HLOC):
                        o = h * MMW + c0
                        nc.vector.tensor_mul(
                            st[:, o:o + 128], st[:, o:o + 128], mask_sb
                        )
                for h in range(HLOC):
                    at = atp.tile([128, MMW], BF16, name="at")
                    nc.scalar.activation(at, st[:, h * MMW:(h + 1) * MMW],
                                         mybir.ActivationFunctionType.Exp)
                    at_tiles[kt, h] = at
                # interleave A@V of an already-exp'd tile between score
                # pairs: keeps the PE busy without pausing the exp stream
                if kt >= shift:
                    av(kt - shift)
                # spread filler PE work evenly across the qt; the exp stream
                # (ACT) is the bottleneck, this rides in PE slack
                if filler and (kt + 1) * nfil // NKT > kt * nfil // NKT:
                    filler.popleft()()
            for kt in range(NKT - shift, NKT):
                av(kt)
            while filler:
                filler.popleft()()
            for h, ct in enumerate(cts):
                rz = rzp.tile([1, MMW], F32, name="rz")
                nc.vector.reciprocal(rz, ct[64:65, :])
                rzb = rzbp.tile([64, MMW], F32, name="rzb")
                nc.gpsimd.partition_broadcast(rzb, rz)
                nc.vector.tensor_mul(
                    C[h * 64:(h + 1) * 64, qt * MMW:(qt + 1) * MMW],
                    ct[0:64, :], rzb,
                )

        def outproj_unit(b, mt, j, opp, osp):
            """One 128-row x 512-col out-projection tile of batch b."""
            C = C_tiles[b]
            op = opp.tile([128, MMW], F32, name="op")
            nc.tensor.matmul(
                op,
                lhsT=C[:, mt * 128:(mt + 1) * 128],
                rhs=Wo_sb[:, j * MMW:(j + 1) * MMW],
                start=True,
                stop=True,
            )
            osb = osp.tile([128, MMW], BF16, name="osb")
            nc.vector.tensor_copy(osb, op)
            nc.sync.dma_start(
                out.ap()[b * M + mt * 128: b * M + (mt + 1) * 128,
                         j * MMW:(j + 1) * MMW],
                osb,
            )

        def outproj_chunk(b, mts, opp, osp):
            for mt in mts:
                for j in range(D // MMW):
                    outproj_unit(b, mt, j, opp, osp)

        # ---------------- the pipelined schedule ----------------
        from collections import deque
        import kernel as _self
        skip_attn = getattr(_self, "ABL_skip_attn", False)
        skip_outproj = getattr(_self, "ABL_skip_outproj", False)
        no_splice = getattr(_self, "ABL_no_splice", False)

        for b in range(B):
            C_tiles[b] = cp.tile([128, M], BF16, name="C")

        with tc.tile_pool(name="proj_psum", bufs=1, space="PSUM") as pp:
            # serial head: projections for batch 0
            for mt_loc in range(NQT):
                proj_chunk(0, mt_loc, pp)
            if skip_attn:
                for mt_loc in range(NQT):
                    proj_chunk(1, mt_loc, pp)
                nc.sync.dma_start(out.ap()[0:128, 0:D], QT[:, 0:D])
                return
            # attention(b0) with proj(b1) spliced into the per-qt PE slack
            for qt in range(NQT):
                if no_splice:
                    attn_qt(0, qt)
                else:
                    attn_qt(0, qt, filler=deque(
                        (lambda ml=qt, w=w: proj_tile(1, ml, w, pp))
                        for w in ("k", "v", "q")))
            if no_splice:
                for mt_loc in range(NQT):
                    proj_chunk(1, mt_loc, pp)

        if skip_outproj:
            for qt in range(NQT):
                attn_qt(1, qt)
            nc.sync.dma_start(out.ap()[0:128, 0:D], C_tiles[0][:, 0:D])
            nc.sync.dma_start(out.ap()[128:256, 0:D], C_tiles[1][:, 0:D])
            return

        with tc.tile_pool(name="op_psum", bufs=1, space="PSUM") as opp, \
             tc.tile_pool(name="out_sbuf", bufs=4) as osp:
            NMT = M // 128
            MPQ = NMT // NQT        # 128-row out-proj tiles per qt slot
            NJ = D // MMW
            # attention(b1) with outproj(b0) + outproj(b1, qt-1) spliced
            for qt in range(NQT):
                fillers = deque(
                    (lambda mt=mt, j=j: outproj_unit(0, mt, j, opp, osp))
                    for mt in range(qt * MPQ, (qt + 1) * MPQ)
                    for j in range(NJ))
                if qt > 0:
                    fillers.extend(
                        (lambda mt=mt, j=j: outproj_unit(1, mt, j, opp, osp))
                        for mt in range((qt - 1) * MPQ, qt * MPQ)
                        for j in range(NJ))
                attn_qt(1, qt, filler=None if no_splice else fillers)
            if no_splice:
                for mt in range(NMT):
                    outproj_chunk(0, (mt,), opp, osp)
                    outproj_chunk(1, (mt,), opp, osp)
            else:
                # tail: final out-proj chunk of batch 1
                outproj_chunk(1, tuple(range((NQT - 1) * MPQ, NMT)), opp, osp)

        if dbg is not None:
            nc.sync.dma_start(dbg["qt"].ap(), QT)
            nc.sync.dma_start(dbg["kt"].ap(), KT)
            nc.sync.dma_start(dbg["v1"].ap(), V1.rearrange("p a b c d -> p (a b c d)"))
            nc.sync.dma_start(dbg["c"].ap()[:, 0:M], C_tiles[0])
            nc.sync.dma_start(dbg["c"].ap()[:, M:S], C_tiles[1])


def build_bass(M, debug=False, reps=1):
    """Build + compile the per-core Bass program (same program on all cores).

    reps > 1 wraps the whole body in an on-device loop — used only for
    timing (amortizes host dispatch overhead over many executions).
    """
    S = B * M
    nc = bacc.Bacc("TRN2", target_bir_lowering=False, debug=False)
    xT = nc.dram_tensor("xT", [D, S], BF16, kind="ExternalInput")
    wqT = nc.dram_tensor("wqT", [D, DLOC], BF16, kind="ExternalInput")
    wkT = nc.dram_tensor("wkT", [D, DLOC], BF16, kind="ExternalInput")
    wvT = nc.dram_tensor("wvT", [D, DLOC], BF16, kind="ExternalInput")
    woT = nc.dram_tensor("woT", [DLOC, D], BF16, kind="ExternalInput")
    bq = nc.dram_tensor("bq", [DLOC, 1], F32, kind="ExternalInput")
    bk = nc.dram_tensor("bk", [DLOC, 1], F32, kind="ExternalInput")
    mask = nc.dram_tensor("mask", [128, 128], F32, kind="ExternalInput")
    out = nc.dram_tensor("out", [S, D], BF16, kind="ExternalOutput")

    dbg = None
    if debug:
        NKT = M // 128
        dbg = {
            "qt": nc.dram_tensor("dbg_qt", [128, S], BF16, kind="ExternalOutput"),
            "kt": nc.dram_tensor("dbg_kt", [128, S], BF16, kind="ExternalOutput"),
            "v1": nc.dram_tensor("dbg_v1", [128, B * NKT * HLOC * 65], BF16,
                                 kind="ExternalOutput"),
            "c": nc.dram_tensor("dbg_c", [128, S], BF16, kind="ExternalOutput"),
        }

    with tile.TileContext(nc) as tc:
        if reps > 1:
            with tc.For_i(0, reps, 1):
                emit_kernel(tc, M, xT, wqT, wkT, wvT, woT, bq, bk, mask, out,
                            dbg=dbg)
        else:
            emit_kernel(tc, M, xT, wqT, wkT, wvT, woT, bq, bk, mask, out,
                        dbg=dbg)
    nc.compile()
    return nc


def make_in_maps(M, x, Wq, bq, Wk, bk, Wv, Wo):
    """Host-side sharding: per-core input dicts."""
    S = B * M
    bf = ml_dtypes.bfloat16
    scale = 1.0 / np.sqrt(DH)
    xT = np.ascontiguousarray(x.reshape(S, D).T).astype(bf)
    mask = (1.0 - np.eye(128, dtype=np.float32))
    in_maps = []
    for c in range(NCORES):
        sl = slice(c * DLOC, (c + 1) * DLOC)
        in_maps.append({
            "xT": xT,
            "wqT": np.ascontiguousarray((Wq[sl] * scale).T).astype(bf),
            "wkT": np.ascontiguousarray(Wk[sl].T).astype(bf),
            "wvT": np.ascontiguousarray(Wv[sl].T).astype(bf),
            "woT": np.ascontiguousarray(Wo[:, sl].T).astype(bf),
            "bq": (bq[sl] * scale).reshape(DLOC, 1).astype(np.float32),
            "bk": bk[sl].reshape(DLOC, 1).astype(np.float32),
            "mask": mask,
        })
    return in_maps


_NC_CACHE = {}


def kernel(x, Wq, bq, Wk, bk, Wv, bv, Wo, bo):
    x = np.asarray(x, dtype=np.float32)
    Wq = np.asarray(Wq, dtype=np.float32)
    bq = np.asarray(bq, dtype=np.float32)
    Wk = np.asarray(Wk, dtype=np.float32)
    bk = np.asarray(bk, dtype=np.float32)
    Wv = np.asarray(Wv, dtype=np.float32)
    bv = np.asarray(bv, dtype=np.float32)
    Wo = np.asarray(Wo, dtype=np.float32)
    bo = np.asarray(bo, dtype=np.float32)

    M = x.shape[1]
    if M not in _NC_CACHE:
        _NC_CACHE[M] = build_bass(M)
    nc = _NC_CACHE[M]

    in_maps = make_in_maps(M, x, Wq, bq, Wk, bk, Wv, Wo)
    res = run_bass_kernel_spmd(nc, in_maps, core_ids=list(range(NCORES)))

    out = np.zeros((B * M, D), np.float64)
    for c in range(NCORES):
        out += res.results[c]["out"].astype(np.float64)
    out = out.astype(np.float32)
    out += bv @ Wo.T + bo          # folded bv/bo contribution
    return out.reshape(B, M, D)
